# revision 33
# baseline (speedup 1.0000x reference)
"""Trainium2 Bass kernel for 2-layer GCN (GCNConv -> relu -> GCNConv -> Linear).

Strategy (8 NeuronCores, SPMD):
  - Nodes padded to NPAD=100352 and dealt (serpentine, by degree) into 784
    blocks of 128 slots; 98 blocks per core.  Each core owns the aggregation
    for its 98 blocks (edge partition by destination).
  - norm factorizes: out = dinv[dst] * sum_e (h*dinv)[src] (self loops are
    plain edges).  Tables T1=((x*dinv)@W1) (bf16) are computed replicated on
    every core; messages are fetched with dma_gather (int16 indices into 4
    slot-major quarter tables, 4 SWDGE queues) and segment-summed via a
    selection-matrix matmul accumulated in PSUM.  Biases are folded into the
    accumulation as rank-1 matmuls (sqrt(deg) outer b); all epilogue
    elementwise work runs on the otherwise-idle Activation engine.
  - Layer-2 table T2=(relu(agg1*dinv)@W2)*dinv is exchanged with an
    AllGather, then the same gather/matmul pass produces h2, and the final
    y^T = Wfc^T@h2^T + bfc is written per block.  Host un-permutes rows.
"""

import os as _env_os
import numpy as np
import ml_dtypes

P = 128
NCORES = 8
NQ = 4
IN_C, HID, OUT_C = 128, 128, 64
# rows per dma_gather call; calls larger than the SWDGE descriptor ring
# (1024) wedge the device
CALL = int(_env_os.environ.get("KB_CALL", "1024"))
GBUFS = int(_env_os.environ.get("KB_GBUFS", str(max(2, 24 * 1024 // CALL))))
NQUEUES = int(_env_os.environ.get("KB_NQUEUES", "4"))


def _set_size(n_nodes, bpc):
    """Set problem size (module-global); mini sizes used by the CoreSim check."""
    global N, BPC, NBINS, NPAD, SHARD, QROWS, QB, BQ, BQ0
    N = n_nodes
    BPC = bpc
    NBINS = NCORES * BPC
    NPAD = NBINS * P
    SHARD = BPC * P
    QROWS = NPAD // NQ
    QB = NBINS // NQ          # T1 blocks per quarter table
    # layer-2 tables are quartered by dst-block range so each AllGather can
    # fire as soon as its block range of phase B completes
    base, rem = BPC // NQ, BPC % NQ
    BQ = [base + (1 if q < rem else 0) for q in range(NQ)]
    BQ0 = [sum(BQ[:q]) for q in range(NQ + 1)]
    assert N <= NPAD and QROWS <= 32768 and NBINS % NQ == 0
    assert all(NCORES * P * b <= 32768 for b in BQ)


_set_size(100000, 98)

_kernel_cache = {}


def _wrap_idx(st):
    """[C, NQ, SLEN] int16 -> [C, NQ, 128, SLEN//16] wrapped+replicated."""
    C, Q, SLEN = st.shape
    w = st.reshape(C, Q, SLEN // 16, 16)
    w = np.swapaxes(w, 2, 3)                       # [C, Q, 16, SLEN//16]
    return np.ascontiguousarray(np.tile(w, (1, 1, 8, 1)))


def _edge_arrays(q, lidx, core, b, dslot):
    """Build per-core gather-index streams and dst-slot arrays for one layer.

    Streams are grouped by (core, quarter, block); each (block, quarter) cell
    is padded to a multiple of 128 lanes (dummy idx 0, dst-slot 255 -> zero
    row in the selection matrix) and sorted by table row for DMA locality."""
    lidx = lidx.astype(np.int16)
    cell = (core * NQ + q) * BPC + b
    ncell = NCORES * NQ * BPC
    counts = np.bincount(cell, minlength=ncell)
    K = int(np.ceil(counts.max() / P))
    CAP = K * P
    order = np.lexsort((lidx, cell))
    start = np.zeros(ncell + 1, np.int64)
    np.cumsum(counts, out=start[1:])
    rank = np.arange(cell.shape[0]) - start[cell[order]]
    pos = cell[order] * CAP + rank
    idx_arr = np.zeros(ncell * CAP, np.int16)
    dl_arr = np.full(ncell * CAP, 255.0, np.float32)
    idx_arr[pos] = lidx[order]
    dl_arr[pos] = dslot[order]
    # pad each (core, quarter) stream to a multiple of CALL so every
    # dma_gather call is a uniform CALL indices (a trailing partial call
    # was observed to wedge the device)
    slen = BPC * CAP
    slen_pad = -(-slen // CALL) * CALL
    st = np.zeros((NCORES, NQ, slen_pad), np.int16)
    st[:, :, :slen] = idx_arr.reshape(NCORES, NQ, slen)
    gidx = _wrap_idx(st)
    dl = dl_arr.reshape(NCORES, NQ, BPC, K, P)
    dl = dl.transpose(0, 4, 2, 1, 3).reshape(NCORES, P, BPC * NQ * K)
    return K, gidx, dl.astype(ml_dtypes.bfloat16)


def _preprocess(x, edge_index, W1, b1, W2, b2, Wfc, bfc):
    src = np.asarray(edge_index[0], dtype=np.int64)
    dst = np.asarray(edge_index[1], dtype=np.int64)
    deg = (np.bincount(dst, minlength=N) + 1).astype(np.float32)
    dinv_pad = np.ones(NPAD, np.float32)
    dinv_pad[:N] = (1.0 / np.sqrt(deg)).astype(np.float32)

    loop = np.arange(N, dtype=np.int64)
    src_a = np.concatenate([src, loop])
    dst_a = np.concatenate([dst, loop])

    # serpentine deal by degree -> (bin, slot); balances per-block edge counts
    key = np.zeros(NPAD, np.float32)
    key[:N] = deg
    order = np.argsort(-key, kind="stable")
    i = np.arange(NPAD)
    r, c = i // NBINS, i % NBINS
    bins_for_rank = np.where(r % 2 == 0, c, NBINS - 1 - c)
    perm_bin = np.empty(NPAD, np.int64)
    perm_slot = np.empty(NPAD, np.int64)
    perm_bin[order] = bins_for_rank
    perm_slot[order] = r
    perm_pos = perm_bin * P + perm_slot
    pos2node = np.empty(NPAD, np.int64)
    pos2node[perm_pos] = np.arange(NPAD)

    ecore = perm_bin[dst_a] // BPC
    eb = perm_bin[dst_a] % BPC
    edslot = perm_slot[dst_a].astype(np.float32)

    # layer 1: quarter = node-id quarter; within a quarter the table is
    # slot-major (row = (n%128)*QB + block-within-quarter) so phase A writes
    # one contiguous span per partition.
    q1 = src_a // QROWS
    lid = src_a % QROWS
    lidx1 = (lid % P) * QB + lid // P
    K1, gidx1, dl1 = _edge_arrays(q1, lidx1, ecore, eb, edslot)
    # layer 2: quarter = dst-block range of the source position; within
    # (core, quarter) the table is slot-major (row = core*(128*BQ[q]) +
    # slot*BQ[q] + (block - BQ0[q])).
    spos = perm_pos[src_a]
    sc, sb_, ss = spos // SHARD, (spos % SHARD) // P, spos % P
    bqs = np.asarray(BQ, np.int64)
    bq0 = np.asarray(BQ0[:NQ], np.int64)
    q2 = np.searchsorted(np.asarray(BQ0[1:], np.int64), sb_, side="right")
    idx2 = sc * (P * bqs[q2]) + ss * bqs[q2] + (sb_ - bq0[q2])
    K2, gidx2, dl2 = _edge_arrays(q2, idx2, ecore, eb, edslot)

    xpad = np.zeros((NPAD, IN_C), np.float32)
    xpad[:N] = np.asarray(x, np.float32) * dinv_pad[:N, None]  # dinv folded in
    xT = np.ascontiguousarray(xpad.T).astype(ml_dtypes.bfloat16)  # [128, NPAD]

    dinv_pos = dinv_pad[pos2node]                            # dinv by position
    dinv_blk = dinv_pos.reshape(NCORES, BPC, P).transpose(0, 2, 1)  # [C,128,98]
    dinvinv = (1.0 / dinv_pos).reshape(NCORES, 1, BPC * P)   # sqrt(deg) rows

    # iota_rep[p, l*NCH + c] = l: the compare runs with every operand's last
    # AP dim packed (stride 1) so the DVE 2x_1p fast mode applies.
    def _iota_rep(K):
        nch = NQ * K
        return np.ascontiguousarray(
            np.tile(np.repeat(np.arange(P, dtype=np.float32), nch)[None, :], (P, 1))
        ).astype(ml_dtypes.bfloat16)

    common = {
        "xT": xT,
        "W1": np.asarray(W1, np.float32).astype(ml_dtypes.bfloat16),
        "W2": np.asarray(W2, np.float32).astype(ml_dtypes.bfloat16),
        "Wfc": np.asarray(Wfc, np.float32).astype(ml_dtypes.bfloat16),
        "b1row": np.asarray(b1, np.float32)[None, :].astype(ml_dtypes.bfloat16),
        "b2row": np.asarray(b2, np.float32)[None, :].astype(ml_dtypes.bfloat16),
        "bfc_col": np.ascontiguousarray(np.asarray(bfc, np.float32)[:, None]),
        "iota1": _iota_rep(K1),
        "iota2": _iota_rep(K2),
    }
    in_maps = []
    for c in range(NCORES):
        m = dict(common)
        m["dinv_blk"] = np.ascontiguousarray(dinv_blk[c])
        m["dinvinv"] = np.ascontiguousarray(dinvinv[c].astype(ml_dtypes.bfloat16))
        m["dstloc1"] = np.ascontiguousarray(dl1[c])
        m["dstloc2"] = np.ascontiguousarray(dl2[c])
        m["gidx1"] = np.ascontiguousarray(gidx1[c])
        m["gidx2"] = np.ascontiguousarray(gidx2[c])
        in_maps.append(m)
    return K1, K2, in_maps, perm_pos


def _build(K1, K2):
    import os
    import concourse.bass as bass  # noqa: F401
    import concourse.mybir as mybir
    import concourse.tile as tile
    from concourse import bacc
    from concourse.masks import make_identity

    stop_after = os.environ.get("KB_STOP_AFTER", "")   # "", "A", "B", "CC"
    dt = mybir.dt
    OP = mybir.AluOpType
    AF = mybir.ActivationFunctionType
    _pad = lambda s: -(-s // CALL) * CALL
    SLEN1, SLEN2 = _pad(BPC * K1 * P), _pad(BPC * K2 * P)
    CPP = CALL // P     # chunks per gather call

    nc = bacc.Bacc("TRN2", num_devices=NCORES, target_bir_lowering=False, debug=False,
                   num_swdge_queues=NQUEUES)

    xT = nc.dram_tensor("xT", [P, NPAD], dt.bfloat16, kind="ExternalInput")
    W1 = nc.dram_tensor("W1", [IN_C, HID], dt.bfloat16, kind="ExternalInput")
    W2 = nc.dram_tensor("W2", [HID, OUT_C], dt.bfloat16, kind="ExternalInput")
    Wfc = nc.dram_tensor("Wfc", [OUT_C, OUT_C], dt.bfloat16, kind="ExternalInput")
    b1row = nc.dram_tensor("b1row", [1, HID], dt.bfloat16, kind="ExternalInput")
    b2row = nc.dram_tensor("b2row", [1, OUT_C], dt.bfloat16, kind="ExternalInput")
    bfc_col = nc.dram_tensor("bfc_col", [OUT_C, 1], dt.float32, kind="ExternalInput")
    iota1 = nc.dram_tensor("iota1", [P, P * NQ * K1], dt.bfloat16, kind="ExternalInput")
    iota2 = nc.dram_tensor("iota2", [P, P * NQ * K2], dt.bfloat16, kind="ExternalInput")
    dinv_blk = nc.dram_tensor("dinv_blk", [P, BPC], dt.float32, kind="ExternalInput")
    dinvinv = nc.dram_tensor("dinvinv", [1, BPC * P], dt.bfloat16, kind="ExternalInput")
    dstloc1 = nc.dram_tensor("dstloc1", [P, BPC * NQ * K1], dt.bfloat16, kind="ExternalInput")
    dstloc2 = nc.dram_tensor("dstloc2", [P, BPC * NQ * K2], dt.bfloat16, kind="ExternalInput")
    gidx1 = nc.dram_tensor("gidx1", [NQ, P, SLEN1 // 16], dt.int16, kind="ExternalInput")
    gidx2 = nc.dram_tensor("gidx2", [NQ, P, SLEN2 // 16], dt.int16, kind="ExternalInput")
    y = nc.dram_tensor("y", [OUT_C, SHARD], dt.float32, kind="ExternalOutput")

    T1q = [nc.dram_tensor(f"T1_{q}", [QROWS, HID], dt.bfloat16) for q in range(NQ)]
    u2q = [nc.dram_tensor(f"u2_{q}", [P * BQ[q], P], dt.bfloat16) for q in range(NQ)]
    T2q = [nc.dram_tensor(f"T2_{q}", [NCORES * P * BQ[q], P], dt.bfloat16)
           for q in range(NQ)]

    def agg_pass(sb_g, sb_s, ps, iota_t, tables, Kc, gidx_t, dstloc_ap, elem,
                 nout, bias_rhs, epilogue, nblocks=BPC):
        """For each of BPC blocks: gather messages (dma_gather per CALL rows,
        per quarter), build the selection matrix, matmul-accumulate in PSUM
        (plus a rank-1 sqrt(deg) x bias fold), then run the epilogue."""
        slen = _pad(BPC * Kc * P)
        ncalls = slen // CALL
        gtiles = {}
        issued = [0] * NQ

        def issue(q, call):
            n = min(CALL, slen - call * CALL)
            gt = sb_g.tile([P, CPP, elem], dt.bfloat16, tag="gbuf")
            nc.gpsimd.dma_gather(
                out_ap=gt[:, : n // P, :],
                in_ap=tables[q],
                idxs_ap=gidx_t[q][:, call * (CALL // 16) : call * (CALL // 16) + n // 16],
                num_idxs=n,
                num_idxs_reg=n,
                elem_size=elem,
                queue_num=q % NQUEUES,
            )
            gtiles[(q, call)] = gt

        gather_only = os.environ.get("KB_GATHER_ONLY", "0") == "1"
        for b in range(nblocks):
            last_call = ((b + 1) * Kc - 1) // CPP
            for q in range(NQ):
                while issued[q] <= last_call and issued[q] < ncalls:
                    issue(q, issued[q])
                    issued[q] += 1
            if gather_only:
                continue
            NCH = NQ * Kc
            # s_all[p, l, c]: l-major so every is_equal operand has a packed
            # last dim (DVE 2x_1p); the matmul lhsT reads column c strided.
            s_all = sb_s.tile([P, P, NCH], dt.bfloat16, tag="sall")
            nc.vector.tensor_tensor(
                out=s_all[:],
                in0=dstloc_ap[:, b * NCH : (b + 1) * NCH].rearrange(
                    "p (a c) -> p a c", a=1
                ).to_broadcast([P, P, NCH]),
                in1=iota_t[:].rearrange("p (l c) -> p l c", c=NCH),
                op=OP.is_equal,
            )
            psum_agg = ps.tile([P, nout], dt.float32, space="PSUM", tag="agg")
            k = 0
            for q in range(NQ):
                for j in range(Kc):
                    g = b * Kc + j
                    gt = gtiles[(q, g // CPP)]
                    nc.tensor.matmul(
                        out=psum_agg[:],
                        lhsT=s_all[:, :, q * Kc + j],
                        rhs=gt[:, g % CPP, :nout],
                        start=(k == 0),
                        stop=False,
                    )
                    k += 1
            nc.tensor.matmul(
                out=psum_agg[:],
                lhsT=gconst["dinvinv"][:, b * P : (b + 1) * P],
                rhs=bias_rhs,
                start=False,
                stop=True,
            )
            epilogue(b, psum_agg)

    with tile.TileContext(nc) as tc:
        with tc.tile_pool(name="const", bufs=1) as cp:
            gconst = {}
            for name, t, shape, dtt in [
                ("W1", W1, [IN_C, HID], dt.bfloat16),
                ("W2", W2, [HID, OUT_C], dt.bfloat16),
                ("Wfc", Wfc, [OUT_C, OUT_C], dt.bfloat16),
                ("b1row", b1row, [1, HID], dt.bfloat16),
                ("b2row", b2row, [1, OUT_C], dt.bfloat16),
                ("bfc_col", bfc_col, [OUT_C, 1], dt.float32),
                ("iota1", iota1, [P, P * NQ * K1], dt.bfloat16),
                ("iota2", iota2, [P, P * NQ * K2], dt.bfloat16),
                ("dinv_blk", dinv_blk, [P, BPC], dt.float32),
                ("dinvinv", dinvinv, [1, BPC * P], dt.bfloat16),
                ("dstloc1", dstloc1, [P, BPC * NQ * K1], dt.bfloat16),
                ("dstloc2", dstloc2, [P, BPC * NQ * K2], dt.bfloat16),
            ]:
                tl = cp.tile(shape, dtt, tag=name)
                nc.sync.dma_start(out=tl[:], in_=t[:])
                gconst[name] = tl
            ident = cp.tile([P, P], dt.float32, tag="ident")
            make_identity(nc, ident[:])
            gidx1_t = []
            for q in range(NQ):
                tl = cp.tile([P, SLEN1 // 16], dt.int16, tag=f"gidx1_{q}")
                nc.sync.dma_start(out=tl[:], in_=gidx1[q])
                gidx1_t.append(tl)
            gidx2_t = []
            for q in range(NQ):
                tl = cp.tile([P, SLEN2 // 16], dt.int16, tag=f"gidx2_{q}")
                nc.sync.dma_start(out=tl[:], in_=gidx2[q])
                gidx2_t.append(tl)

            # ---------------- phase A: T1 = (x*dinv) @ W1 ------------------
            for _rep in range(int(os.environ.get("KB_REPEAT", "1"))):
                # node-blocks per DMA panel (largest divisor of QB)
                PBLK = next(d for d in (28, 14, 7, 4, 2, 1) if QB % d == 0)
                with (
                    tc.tile_pool(name="phA", bufs=3) as pA,
                    tc.tile_pool(name="psA", bufs=3, space="PSUM") as psA,
                ):
                    for q in range(NQ):
                        for j0 in range(0, QB, PBLK):
                            nb0 = q * QB + j0
                            xt = pA.tile([P, PBLK * P], dt.bfloat16, tag="xpanel")
                            nc.sync.dma_start(
                                out=xt[:], in_=xT[:, nb0 * P : (nb0 + PBLK) * P]
                            )
                            stg = pA.tile([P, PBLK * HID], dt.bfloat16, tag="stg")
                            for g in range(0, PBLK, 4):
                                kk = min(4, PBLK - g)
                                # full-bank psum tile so the start=True zero
                                # region never touches a neighboring tile
                                pt = psA.tile([P, 4 * HID], dt.float32,
                                              space="PSUM", tag="pA")
                                for k in range(kk):
                                    nc.tensor.matmul(
                                        out=pt[:, k * HID : (k + 1) * HID],
                                        lhsT=xt[:, (g + k) * P : (g + k + 1) * P],
                                        rhs=gconst["W1"][:],
                                        start=(k == 0),
                                        stop=(k == kk - 1),
                                    )
                                nc.scalar.copy(
                                    out=stg[:, g * HID : (g + kk) * HID],
                                    in_=pt[:, : kk * HID],
                                )
                            # alternate the HWDGE issue engine so xt reads
                            # (SP) and T1 writes (ACT/SP) overlap
                            eng = nc.scalar if (j0 // PBLK) % 2 == 0 else nc.sync
                            eng.dma_start(
                                out=T1q[q].ap().rearrange(
                                    "(p j) h -> p j h", p=P
                                )[:, j0 : j0 + PBLK, :],
                                in_=stg[:].rearrange("p (k h) -> p k h", h=HID),
                            )

                tc.strict_bb_all_engine_barrier()

                # ---------------- phase B: layer-1 aggregation + u2 --------------
                if stop_after == "A":
                    # debug: stop after phase A; emit a dummy y write
                    with tc.tile_pool(name="dbg", bufs=1) as dbg:
                        z = dbg.tile([OUT_C, SHARD], dt.float32, tag="z")
                        nc.vector.memset(z[:], 0)
                        nc.sync.dma_start(out=y.ap(), in_=z[:])
                run_b = stop_after != "A"
                if run_b:
                  with (
                    tc.tile_pool(name="phB", bufs=GBUFS) as pB,
                    tc.tile_pool(name="phBs", bufs=2) as pBs,
                    tc.tile_pool(name="phBe", bufs=3) as pBe,
                    tc.tile_pool(name="psB", bufs=2, space="PSUM") as psB,
                    tc.tile_pool(name="psBa", bufs=3, space="PSUM") as psBa,
                    tc.tile_pool(name="u2p", bufs=1) as u2pool,
                  ):
                    u2panel = u2pool.tile([P, BPC * P], dt.bfloat16, tag="u2panel")
                    nc.vector.memset(u2panel[:], 0)

                    nb_b = int(os.environ.get("KB_B_BLOCKS", BPC))
                    run_cc = stop_after not in ("A", "B") and nb_b == BPC

                    def epi1(b, psum_agg):
                        h1 = pBe.tile([P, HID], dt.float32, tag="h1")
                        nc.scalar.activation(
                            out=h1[:], in_=psum_agg[:], func=AF.Relu,
                            scale=gconst["dinv_blk"][:, b : b + 1],
                        )
                        ptr = psB.tile([P, P], dt.float32, space="PSUM", tag="tr")
                        nc.tensor.transpose(out=ptr[:], in_=h1[:], identity=ident[:])
                        h1t = pBe.tile([P, P], dt.bfloat16, tag="h1t")
                        nc.scalar.copy(out=h1t[:], in_=ptr[:])
                        pu = psB.tile([P, OUT_C], dt.float32, space="PSUM", tag="pu")
                        nc.tensor.matmul(
                            out=pu[:], lhsT=h1t[:], rhs=gconst["W2"][:],
                            start=True, stop=True,
                        )
                        nc.scalar.activation(
                            out=u2panel[:, b * P : b * P + OUT_C], in_=pu[:],
                            func=AF.Copy, scale=gconst["dinv_blk"][:, b : b + 1],
                        )
                        # as soon as a block range completes, ship it and
                        # start its AllGather (overlaps the rest of phase B)
                        if run_cc and b + 1 in BQ0[1:]:
                            q = BQ0[1:].index(b + 1)
                            nc.sync.dma_start(
                                out=u2q[q].ap().rearrange(
                                    "(s b) h -> s b h", b=BQ[q]
                                ),
                                in_=u2panel[:, BQ0[q] * P : BQ0[q + 1] * P].rearrange(
                                    "p (b h) -> p b h", h=P
                                ),
                            )
                            nc.gpsimd.collective_compute(
                                "AllGather",
                                mybir.AluOpType.bypass,
                                replica_groups=[list(range(NCORES))],
                                ins=[u2q[q].ap()],
                                outs=[T2q[q].ap()],
                            )

                    agg_pass(pB, pBs, psBa, gconst["iota1"],
                             [T1q[q][:] for q in range(NQ)],
                             K1, gidx1_t, gconst["dstloc1"][:], HID, HID,
                             gconst["b1row"][:], epi1, nblocks=nb_b)

                tc.strict_bb_all_engine_barrier()
                run_d = stop_after not in ("A", "B", "CC") and run_cc
                if run_b and not run_d:
                    with tc.tile_pool(name="dbgB", bufs=1) as dbg:
                        z = dbg.tile([OUT_C, SHARD], dt.float32, tag="zB")
                        nc.vector.memset(z[:], 0)
                        nc.sync.dma_start(out=y.ap(), in_=z[:])

                # ---------------- phase D: layer-2 aggregation + FC --------------
                if run_d:
                  with (
                    tc.tile_pool(name="phD", bufs=GBUFS) as pD,
                    tc.tile_pool(name="phDs", bufs=2) as pDs,
                    tc.tile_pool(name="phDe", bufs=3) as pDe,
                    tc.tile_pool(name="psD", bufs=2, space="PSUM") as psD,
                    tc.tile_pool(name="psDa", bufs=3, space="PSUM") as psDa,
                    tc.tile_pool(name="ypl", bufs=2) as ypool,
                ):
                    # y is written in quarter panels to bound SBUF footprint
                    HB = (BPC + 3) // 4
                    ndb = int(os.environ.get("KB_D_BLOCKS", BPC))
                    ystate = {}

                    def epi2(b, psum_agg):
                        if b % HB == 0:
                            ypanel = ypool.tile(
                                [OUT_C, HB * P], dt.float32, tag="ypanel"
                            )
                            ystate["t"] = ypanel
                            ystate["b0"] = b
                        h2 = pDe.tile([P, OUT_C], dt.float32, tag="h2")
                        nc.scalar.activation(
                            out=h2[:], in_=psum_agg[:], func=AF.Copy,
                            scale=gconst["dinv_blk"][:, b : b + 1],
                        )
                        ptr = psD.tile([OUT_C, P], dt.float32, space="PSUM", tag="tr2")
                        nc.tensor.transpose(out=ptr[:], in_=h2[:], identity=ident[:])
                        h2t = pDe.tile([OUT_C, P], dt.bfloat16, tag="h2t")
                        nc.scalar.copy(out=h2t[:], in_=ptr[:])
                        py = psD.tile([OUT_C, P], dt.float32, space="PSUM", tag="py")
                        nc.tensor.matmul(
                            out=py[:], lhsT=gconst["Wfc"][:], rhs=h2t[:],
                            start=True, stop=True,
                        )
                        b0 = ystate["b0"]
                        nc.scalar.activation(
                            out=ystate["t"][:, (b - b0) * P : (b - b0 + 1) * P],
                            in_=py[:],
                            func=AF.Identity, bias=gconst["bfc_col"][:, 0:1],
                        )
                        if b % HB == HB - 1 or b == ndb - 1:
                            nc.sync.dma_start(
                                out=y.ap()[:, b0 * P : (b + 1) * P],
                                in_=ystate["t"][:, : (b - b0 + 1) * P],
                            )

                    agg_pass(pD, pDs, psDa, gconst["iota2"],
                             [T2q[q][:] for q in range(NQ)],
                             K2, gidx2_t, gconst["dstloc2"][:], P, OUT_C,
                             gconst["b2row"][:], epi2, nblocks=ndb)

    nc.compile()
    return nc


def _make_runner(nc):
    """jit-compiled SPMD runner over 8 cores (mirrors bass2jax.run_bass_via_pjrt
    but reusable across calls so executions can be timed warm)."""
    import jax
    import numpy as np
    from jax.sharding import Mesh, PartitionSpec
    from jax.experimental.shard_map import shard_map
    import concourse.mybir as mybir
    from concourse import bass2jax

    bass2jax.install_neuronx_cc_hook()
    partition_name = nc.partition_id_tensor.name if nc.partition_id_tensor else None
    in_names, out_names, out_avals, zero_outs = [], [], [], []
    for alloc in nc.m.functions[0].allocations:
        if not isinstance(alloc, mybir.MemoryLocationSet):
            continue
        name = alloc.memorylocations[0].name
        if alloc.kind == "ExternalInput":
            if name != partition_name:
                in_names.append(name)
        elif alloc.kind == "ExternalOutput":
            out_names.append(name)
            shape = tuple(alloc.tensor_shape)
            dtype = mybir.dt.np(alloc.dtype)
            out_avals.append(jax.core.ShapedArray(shape, dtype))
            zero_outs.append(np.zeros(shape, dtype))
    n_params = len(in_names)
    all_in_names = list(in_names) + list(out_names)
    if partition_name is not None:
        all_in_names.append(partition_name)

    def _body(*args):
        operands = list(args)
        if partition_name is not None:
            operands.append(bass2jax.partition_id_tensor())
        outs = bass2jax._bass_exec_p.bind(
            *operands,
            out_avals=tuple(out_avals),
            in_names=tuple(all_in_names),
            out_names=tuple(out_names),
            lowering_input_output_aliases=(),
            sim_require_finite=True,
            sim_require_nnan=True,
            nc=nc,
        )
        return tuple(outs)

    devices = jax.devices()[:NCORES]
    mesh = Mesh(np.asarray(devices), ("core",))
    in_specs = (PartitionSpec("core"),) * (n_params + len(out_names))
    out_specs = (PartitionSpec("core"),) * len(out_names)
    fn = jax.jit(
        shard_map(_body, mesh=mesh, in_specs=in_specs, out_specs=out_specs,
                  check_rep=False),
        keep_unused=True,
    )
    return fn, in_names, out_names, zero_outs, mesh


def kernel(x, edge_index, W1, b1, W2, b2, Wfc, bfc, _trace=False, _bench=True):
    import time as _time
    import jax
    from jax.sharding import NamedSharding, PartitionSpec

    import os as _os
    K1, K2, in_maps, perm_pos = _preprocess(x, edge_index, W1, b1, W2, b2, Wfc, bfc)
    key = (K1, K2, _os.environ.get("KB_REPEAT", "1"),
           _os.environ.get("KB_STOP_AFTER", ""), _os.environ.get("KB_D_BLOCKS", ""),
           _os.environ.get("KB_B_BLOCKS", ""), _os.environ.get("KB_SHARED_T2", "0"),
           _os.environ.get("KB_GATHER_ONLY", "0"))
    if key not in _kernel_cache:
        nc = _build(K1, K2)
        _kernel_cache[key] = (nc, _make_runner(nc))
    nc, (fn, in_names, out_names, zero_outs, mesh) = _kernel_cache[key]

    sh = NamedSharding(mesh, PartitionSpec("core"))
    concat_in = [
        np.concatenate([np.asarray(in_maps[c][nm]) for c in range(NCORES)], axis=0)
        for nm in in_names
    ]
    concat_zeros = [
        np.zeros((NCORES * z.shape[0], *z.shape[1:]), z.dtype) for z in zero_outs
    ]
    dev_in = [jax.device_put(a, sh) for a in concat_in + concat_zeros]
    out_arrs = fn(*dev_in)
    jax.block_until_ready(out_arrs)

    if _bench:
        times = []
        for _ in range(5):
            t0 = _time.perf_counter()
            out_arrs = fn(*dev_in)
            jax.block_until_ready(out_arrs)
            times.append(_time.perf_counter() - t0)
        kernel._last_times = times
        kernel._last_exec_time_ns = int(min(times) * 1e9)
    else:
        kernel._last_exec_time_ns = None
    if not hasattr(kernel, "_runners"):
        kernel._runners = {}
    kernel._runners[_os.environ.get("KB_REPEAT", "1")] = (fn, dev_in)

    outs = {nm: np.asarray(out_arrs[i]) for i, nm in enumerate(out_names)}
    Y = (
        outs["y"].reshape(NCORES, OUT_C, SHARD)
        .transpose(0, 2, 1)
        .reshape(NCORES * SHARD, OUT_C)
    )
    return Y[perm_pos[:N]].astype(np.float32)


# revision 38
# speedup vs baseline: 3.8679x; 3.8679x over previous
"""Trainium2 Bass kernel for 2-layer GCN (GCNConv -> relu -> GCNConv -> Linear).

Strategy (8 NeuronCores, SPMD):
  - Nodes padded to NPAD=100352 and dealt (serpentine, by degree) into 784
    blocks of 128 slots; 98 blocks per core.  Each core owns the aggregation
    for its 98 blocks (edge partition by destination).
  - norm factorizes: out = dinv[dst] * sum_e (h*dinv)[src] (self loops are
    plain edges).  Tables T1=((x*dinv)@W1) (bf16) are computed replicated on
    every core; messages are fetched with dma_gather (int16 indices into 4
    slot-major quarter tables, 4 SWDGE queues) and segment-summed via a
    selection-matrix matmul accumulated in PSUM.  Biases are folded into the
    accumulation as rank-1 matmuls (sqrt(deg) outer b); all epilogue
    elementwise work runs on the otherwise-idle Activation engine.
  - Layer-2 table T2=(relu(agg1*dinv)@W2)*dinv is exchanged with an
    AllGather, then the same gather/matmul pass produces h2, and the final
    y^T = Wfc^T@h2^T + bfc is written per block.  Host un-permutes rows.
"""

import os as _env_os
import numpy as np
import ml_dtypes

P = 128
NCORES = 8
NQ = 4
IN_C, HID, OUT_C = 128, 128, 64
# rows per dma_gather call; calls larger than the SWDGE descriptor ring
# (1024) wedge the device
CALL = int(_env_os.environ.get("KB_CALL", "1024"))
GBUFS = int(_env_os.environ.get("KB_GBUFS", str(max(2, 24 * 1024 // CALL))))
NQUEUES = int(_env_os.environ.get("KB_NQUEUES", "4"))


def _set_size(n_nodes, bpc):
    """Set problem size (module-global); mini sizes used by the CoreSim check."""
    global N, BPC, NBINS, NPAD, SHARD, QROWS, QB, PS
    N = n_nodes
    BPC = bpc
    NBINS = NCORES * BPC
    NPAD = NBINS * P
    SHARD = BPC * P
    QROWS = NPAD // NQ
    QB = NBINS // NQ          # T1 blocks per quarter table
    PS = P // NQ              # u2 slots per quarter table
    assert N <= NPAD and QROWS <= 32768 and NBINS % NQ == 0


_set_size(100000, 98)

_kernel_cache = {}


def _wrap_idx(st):
    """[C, NQ, SLEN] int16 -> [C, NQ, 128, SLEN//16] wrapped+replicated."""
    C, Q, SLEN = st.shape
    w = st.reshape(C, Q, SLEN // 16, 16)
    w = np.swapaxes(w, 2, 3)                       # [C, Q, 16, SLEN//16]
    return np.ascontiguousarray(np.tile(w, (1, 1, 8, 1)))


def _edge_arrays(q, lidx, core, b, dslot):
    """Build per-core gather-index streams and dst-slot arrays for one layer.

    Streams are grouped by (core, quarter, block); each (block, quarter) cell
    is padded to a multiple of 128 lanes (dummy idx 0, dst-slot 255 -> zero
    row in the selection matrix) and sorted by table row for DMA locality."""
    lidx = lidx.astype(np.int16)
    cell = (core * NQ + q) * BPC + b
    ncell = NCORES * NQ * BPC
    counts = np.bincount(cell, minlength=ncell)
    K = int(np.ceil(counts.max() / P))
    CAP = K * P
    order = np.lexsort((lidx, cell))
    start = np.zeros(ncell + 1, np.int64)
    np.cumsum(counts, out=start[1:])
    rank = np.arange(cell.shape[0]) - start[cell[order]]
    pos = cell[order] * CAP + rank
    idx_arr = np.zeros(ncell * CAP, np.int16)
    dl_arr = np.full(ncell * CAP, 255.0, np.float32)
    idx_arr[pos] = lidx[order]
    dl_arr[pos] = dslot[order]
    # pad each (core, quarter) stream to a multiple of CALL so every
    # dma_gather call is a uniform CALL indices (a trailing partial call
    # was observed to wedge the device)
    slen = BPC * CAP
    slen_pad = -(-slen // CALL) * CALL
    st = np.zeros((NCORES, NQ, slen_pad), np.int16)
    st[:, :, :slen] = idx_arr.reshape(NCORES, NQ, slen)
    gidx = _wrap_idx(st)
    dl = dl_arr.reshape(NCORES, NQ, BPC, K, P)
    dl = dl.transpose(0, 4, 2, 1, 3).reshape(NCORES, P, BPC * NQ * K)
    return K, gidx, dl.astype(ml_dtypes.bfloat16)


def _preprocess(x, edge_index, W1, b1, W2, b2, Wfc, bfc):
    src = np.asarray(edge_index[0], dtype=np.int64)
    dst = np.asarray(edge_index[1], dtype=np.int64)
    deg = (np.bincount(dst, minlength=N) + 1).astype(np.float32)
    dinv_pad = np.ones(NPAD, np.float32)
    dinv_pad[:N] = (1.0 / np.sqrt(deg)).astype(np.float32)

    loop = np.arange(N, dtype=np.int64)
    src_a = np.concatenate([src, loop])
    dst_a = np.concatenate([dst, loop])

    # serpentine deal by degree -> (bin, slot); balances per-block edge counts
    key = np.zeros(NPAD, np.float32)
    key[:N] = deg
    order = np.argsort(-key, kind="stable")
    i = np.arange(NPAD)
    r, c = i // NBINS, i % NBINS
    bins_for_rank = np.where(r % 2 == 0, c, NBINS - 1 - c)
    perm_bin = np.empty(NPAD, np.int64)
    perm_slot = np.empty(NPAD, np.int64)
    perm_bin[order] = bins_for_rank
    perm_slot[order] = r
    perm_pos = perm_bin * P + perm_slot
    pos2node = np.empty(NPAD, np.int64)
    pos2node[perm_pos] = np.arange(NPAD)

    ecore = perm_bin[dst_a] // BPC
    eb = perm_bin[dst_a] % BPC
    edslot = perm_slot[dst_a].astype(np.float32)

    # layer 1: quarter = node-id quarter; within a quarter the table is
    # slot-major (row = (n%128)*QB + block-within-quarter) so phase A writes
    # one contiguous span per partition.
    q1 = src_a // QROWS
    lid = src_a % QROWS
    lidx1 = (lid % P) * QB + lid // P
    K1, gidx1, dl1 = _edge_arrays(q1, lidx1, ecore, eb, edslot)
    # layer 2: quarter = slot//32; within (core, quarter) the table is
    # slot-major (row = core*(SHARD//NQ) + (slot%32)*BPC + block).
    spos = perm_pos[src_a]
    sc, sb_, ss = spos // SHARD, (spos % SHARD) // P, spos % P
    q2 = ss // PS
    idx2 = sc * (SHARD // NQ) + (ss % PS) * BPC + sb_
    K2, gidx2, dl2 = _edge_arrays(q2, idx2, ecore, eb, edslot)

    xpad = np.zeros((NPAD, IN_C), np.float32)
    xpad[:N] = np.asarray(x, np.float32) * dinv_pad[:N, None]  # dinv folded in
    xT = np.ascontiguousarray(xpad.T).astype(ml_dtypes.bfloat16)  # [128, NPAD]

    dinv_pos = dinv_pad[pos2node]                            # dinv by position
    dinv_blk = dinv_pos.reshape(NCORES, BPC, P).transpose(0, 2, 1)  # [C,128,98]
    dinvinv = (1.0 / dinv_pos).reshape(NCORES, 1, BPC * P)   # sqrt(deg) rows

    # iota_rep[p, l*NCH + c] = l: the compare runs with every operand's last
    # AP dim packed (stride 1) so the DVE 2x_1p fast mode applies.
    def _iota_rep(K):
        nch = NQ * K
        return np.ascontiguousarray(
            np.tile(np.repeat(np.arange(P, dtype=np.float32), nch)[None, :], (P, 1))
        ).astype(ml_dtypes.bfloat16)

    common = {
        "xT": xT,
        "W1": np.asarray(W1, np.float32).astype(ml_dtypes.bfloat16),
        "W2": np.asarray(W2, np.float32).astype(ml_dtypes.bfloat16),
        "Wfc": np.asarray(Wfc, np.float32).astype(ml_dtypes.bfloat16),
        "b1row": np.asarray(b1, np.float32)[None, :].astype(ml_dtypes.bfloat16),
        "b2row": np.asarray(b2, np.float32)[None, :].astype(ml_dtypes.bfloat16),
        "bfc_col": np.ascontiguousarray(np.asarray(bfc, np.float32)[:, None]),
        "iota1": _iota_rep(K1),
        "iota2": _iota_rep(K2),
    }
    in_maps = []
    for c in range(NCORES):
        m = dict(common)
        m["dinv_blk"] = np.ascontiguousarray(dinv_blk[c])
        m["dinvinv"] = np.ascontiguousarray(dinvinv[c].astype(ml_dtypes.bfloat16))
        m["dstloc1"] = np.ascontiguousarray(dl1[c])
        m["dstloc2"] = np.ascontiguousarray(dl2[c])
        m["gidx1"] = np.ascontiguousarray(gidx1[c])
        m["gidx2"] = np.ascontiguousarray(gidx2[c])
        in_maps.append(m)
    return K1, K2, in_maps, perm_pos


def _build(K1, K2):
    import os
    import concourse.bass as bass  # noqa: F401
    import concourse.mybir as mybir
    import concourse.tile as tile
    from concourse import bacc
    from concourse.masks import make_identity

    stop_after = os.environ.get("KB_STOP_AFTER", "")   # "", "A", "B", "CC"
    dt = mybir.dt
    OP = mybir.AluOpType
    AF = mybir.ActivationFunctionType
    _pad = lambda s: -(-s // CALL) * CALL
    SLEN1, SLEN2 = _pad(BPC * K1 * P), _pad(BPC * K2 * P)
    CPP = CALL // P     # chunks per gather call

    nc = bacc.Bacc("TRN2", num_devices=NCORES, target_bir_lowering=False, debug=False,
                   num_swdge_queues=NQUEUES)

    xT = nc.dram_tensor("xT", [P, NPAD], dt.bfloat16, kind="ExternalInput")
    W1 = nc.dram_tensor("W1", [IN_C, HID], dt.bfloat16, kind="ExternalInput")
    W2 = nc.dram_tensor("W2", [HID, OUT_C], dt.bfloat16, kind="ExternalInput")
    Wfc = nc.dram_tensor("Wfc", [OUT_C, OUT_C], dt.bfloat16, kind="ExternalInput")
    b1row = nc.dram_tensor("b1row", [1, HID], dt.bfloat16, kind="ExternalInput")
    b2row = nc.dram_tensor("b2row", [1, OUT_C], dt.bfloat16, kind="ExternalInput")
    bfc_col = nc.dram_tensor("bfc_col", [OUT_C, 1], dt.float32, kind="ExternalInput")
    iota1 = nc.dram_tensor("iota1", [P, P * NQ * K1], dt.bfloat16, kind="ExternalInput")
    iota2 = nc.dram_tensor("iota2", [P, P * NQ * K2], dt.bfloat16, kind="ExternalInput")
    dinv_blk = nc.dram_tensor("dinv_blk", [P, BPC], dt.float32, kind="ExternalInput")
    dinvinv = nc.dram_tensor("dinvinv", [1, BPC * P], dt.bfloat16, kind="ExternalInput")
    dstloc1 = nc.dram_tensor("dstloc1", [P, BPC * NQ * K1], dt.bfloat16, kind="ExternalInput")
    dstloc2 = nc.dram_tensor("dstloc2", [P, BPC * NQ * K2], dt.bfloat16, kind="ExternalInput")
    gidx1 = nc.dram_tensor("gidx1", [NQ, P, SLEN1 // 16], dt.int16, kind="ExternalInput")
    gidx2 = nc.dram_tensor("gidx2", [NQ, P, SLEN2 // 16], dt.int16, kind="ExternalInput")
    y = nc.dram_tensor("y", [OUT_C, SHARD], dt.float32, kind="ExternalOutput")

    T1q = [nc.dram_tensor(f"T1_{q}", [QROWS, HID], dt.bfloat16) for q in range(NQ)]
    u2loc = nc.dram_tensor("u2loc", [NQ, SHARD // NQ, P], dt.bfloat16)
    T2q = [nc.dram_tensor(f"T2_{q}", [QROWS, P], dt.bfloat16) for q in range(NQ)]

    def agg_pass(sb_g, sb_s, ps, iota_t, tables, Kc, gidx_t, dstloc_ap, elem,
                 nout, bias_rhs, epilogue, nblocks=BPC):
        """For each of BPC blocks: gather messages (dma_gather per CALL rows,
        per quarter), build the selection matrix, matmul-accumulate in PSUM
        (plus a rank-1 sqrt(deg) x bias fold), then run the epilogue."""
        slen = _pad(BPC * Kc * P)
        ncalls = slen // CALL
        gtiles = {}
        issued = [0] * NQ

        def issue(q, call):
            n = min(CALL, slen - call * CALL)
            gt = sb_g.tile([P, CPP, elem], dt.bfloat16, tag="gbuf")
            nc.gpsimd.dma_gather(
                out_ap=gt[:, : n // P, :],
                in_ap=tables[q],
                idxs_ap=gidx_t[q][:, call * (CALL // 16) : call * (CALL // 16) + n // 16],
                num_idxs=n,
                num_idxs_reg=n,
                elem_size=elem,
                queue_num=q % NQUEUES,
            )
            gtiles[(q, call)] = gt

        gather_only = os.environ.get("KB_GATHER_ONLY", "0") == "1"
        for b in range(nblocks):
            last_call = ((b + 1) * Kc - 1) // CPP
            for q in range(NQ):
                while issued[q] <= last_call and issued[q] < ncalls:
                    issue(q, issued[q])
                    issued[q] += 1
            if gather_only:
                continue
            NCH = NQ * Kc
            # s_all[p, l, c]: l-major so every is_equal operand has a packed
            # last dim (DVE 2x_1p); the matmul lhsT reads column c strided.
            s_all = sb_s.tile([P, P, NCH], dt.bfloat16, tag="sall")
            nc.vector.tensor_tensor(
                out=s_all[:],
                in0=dstloc_ap[:, b * NCH : (b + 1) * NCH].rearrange(
                    "p (a c) -> p a c", a=1
                ).to_broadcast([P, P, NCH]),
                in1=iota_t[:].rearrange("p (l c) -> p l c", c=NCH),
                op=OP.is_equal,
            )
            psum_agg = ps.tile([P, nout], dt.float32, space="PSUM", tag="agg")
            k = 0
            for q in range(NQ):
                for j in range(Kc):
                    g = b * Kc + j
                    gt = gtiles[(q, g // CPP)]
                    nc.tensor.matmul(
                        out=psum_agg[:],
                        lhsT=s_all[:, :, q * Kc + j],
                        rhs=gt[:, g % CPP, :nout],
                        start=(k == 0),
                        stop=False,
                    )
                    k += 1
            nc.tensor.matmul(
                out=psum_agg[:],
                lhsT=gconst["dinvinv"][:, b * P : (b + 1) * P],
                rhs=bias_rhs,
                start=False,
                stop=True,
            )
            epilogue(b, psum_agg)

    with tile.TileContext(nc) as tc:
        with tc.tile_pool(name="const", bufs=1) as cp:
            gconst = {}
            for name, t, shape, dtt in [
                ("W1", W1, [IN_C, HID], dt.bfloat16),
                ("W2", W2, [HID, OUT_C], dt.bfloat16),
                ("Wfc", Wfc, [OUT_C, OUT_C], dt.bfloat16),
                ("b1row", b1row, [1, HID], dt.bfloat16),
                ("b2row", b2row, [1, OUT_C], dt.bfloat16),
                ("bfc_col", bfc_col, [OUT_C, 1], dt.float32),
                ("iota1", iota1, [P, P * NQ * K1], dt.bfloat16),
                ("iota2", iota2, [P, P * NQ * K2], dt.bfloat16),
                ("dinv_blk", dinv_blk, [P, BPC], dt.float32),
                ("dinvinv", dinvinv, [1, BPC * P], dt.bfloat16),
                ("dstloc1", dstloc1, [P, BPC * NQ * K1], dt.bfloat16),
                ("dstloc2", dstloc2, [P, BPC * NQ * K2], dt.bfloat16),
            ]:
                tl = cp.tile(shape, dtt, tag=name)
                nc.sync.dma_start(out=tl[:], in_=t[:])
                gconst[name] = tl
            ident = cp.tile([P, P], dt.float32, tag="ident")
            make_identity(nc, ident[:])
            gidx1_t = []
            for q in range(NQ):
                tl = cp.tile([P, SLEN1 // 16], dt.int16, tag=f"gidx1_{q}")
                nc.sync.dma_start(out=tl[:], in_=gidx1[q])
                gidx1_t.append(tl)
            gidx2_t = []
            for q in range(NQ):
                tl = cp.tile([P, SLEN2 // 16], dt.int16, tag=f"gidx2_{q}")
                nc.sync.dma_start(out=tl[:], in_=gidx2[q])
                gidx2_t.append(tl)

            # ---------------- phase A: T1 = (x*dinv) @ W1 ------------------
            for _rep in range(int(os.environ.get("KB_REPEAT", "1"))):
                # node-blocks per DMA panel (largest divisor of QB)
                PBLK = next(d for d in (28, 14, 7, 4, 2, 1) if QB % d == 0)
                with (
                    tc.tile_pool(name="phA", bufs=3) as pA,
                    tc.tile_pool(name="psA", bufs=3, space="PSUM") as psA,
                ):
                    for q in range(NQ):
                        for j0 in range(0, QB, PBLK):
                            nb0 = q * QB + j0
                            xt = pA.tile([P, PBLK * P], dt.bfloat16, tag="xpanel")
                            nc.sync.dma_start(
                                out=xt[:], in_=xT[:, nb0 * P : (nb0 + PBLK) * P]
                            )
                            stg = pA.tile([P, PBLK * HID], dt.bfloat16, tag="stg")
                            for g in range(0, PBLK, 4):
                                kk = min(4, PBLK - g)
                                # full-bank psum tile so the start=True zero
                                # region never touches a neighboring tile
                                pt = psA.tile([P, 4 * HID], dt.float32,
                                              space="PSUM", tag="pA")
                                for k in range(kk):
                                    nc.tensor.matmul(
                                        out=pt[:, k * HID : (k + 1) * HID],
                                        lhsT=xt[:, (g + k) * P : (g + k + 1) * P],
                                        rhs=gconst["W1"][:],
                                        start=(k == 0),
                                        stop=(k == kk - 1),
                                    )
                                nc.scalar.copy(
                                    out=stg[:, g * HID : (g + kk) * HID],
                                    in_=pt[:, : kk * HID],
                                )
                            # alternate the HWDGE issue engine so xt reads
                            # (SP) and T1 writes (ACT/SP) overlap
                            eng = nc.scalar if (j0 // PBLK) % 2 == 0 else nc.sync
                            eng.dma_start(
                                out=T1q[q].ap().rearrange(
                                    "(p j) h -> p j h", p=P
                                )[:, j0 : j0 + PBLK, :],
                                in_=stg[:].rearrange("p (k h) -> p k h", h=HID),
                            )

                tc.strict_bb_all_engine_barrier()

                # ---------------- phase B: layer-1 aggregation + u2 --------------
                if stop_after == "A":
                    # debug: stop after phase A; emit a dummy y write
                    with tc.tile_pool(name="dbg", bufs=1) as dbg:
                        z = dbg.tile([OUT_C, SHARD], dt.float32, tag="z")
                        nc.vector.memset(z[:], 0)
                        nc.sync.dma_start(out=y.ap(), in_=z[:])
                run_b = stop_after != "A"
                if run_b:
                  with (
                    tc.tile_pool(name="phB", bufs=GBUFS) as pB,
                    tc.tile_pool(name="phBs", bufs=2) as pBs,
                    tc.tile_pool(name="phBe", bufs=3) as pBe,
                    tc.tile_pool(name="psB", bufs=2, space="PSUM") as psB,
                    tc.tile_pool(name="psBa", bufs=3, space="PSUM") as psBa,
                    tc.tile_pool(name="u2p", bufs=1) as u2pool,
                  ):
                    u2panel = u2pool.tile([P, BPC * P], dt.bfloat16, tag="u2panel")
                    nc.vector.memset(u2panel[:], 0)

                    def epi1(b, psum_agg):
                        h1 = pBe.tile([P, HID], dt.float32, tag="h1")
                        nc.scalar.activation(
                            out=h1[:], in_=psum_agg[:], func=AF.Relu,
                            scale=gconst["dinv_blk"][:, b : b + 1],
                        )
                        ptr = psB.tile([P, P], dt.float32, space="PSUM", tag="tr")
                        nc.tensor.transpose(out=ptr[:], in_=h1[:], identity=ident[:])
                        h1t = pBe.tile([P, P], dt.bfloat16, tag="h1t")
                        nc.scalar.copy(out=h1t[:], in_=ptr[:])
                        pu = psB.tile([P, OUT_C], dt.float32, space="PSUM", tag="pu")
                        nc.tensor.matmul(
                            out=pu[:], lhsT=h1t[:], rhs=gconst["W2"][:],
                            start=True, stop=True,
                        )
                        nc.scalar.activation(
                            out=u2panel[:, b * P : b * P + OUT_C], in_=pu[:],
                            func=AF.Copy, scale=gconst["dinv_blk"][:, b : b + 1],
                        )

                    agg_pass(pB, pBs, psBa, gconst["iota1"],
                             [T1q[q][:] for q in range(NQ)],
                             K1, gidx1_t, gconst["dstloc1"][:], HID, HID,
                             gconst["b1row"][:], epi1,
                             nblocks=int(os.environ.get("KB_B_BLOCKS", BPC)))
                    # u2 panel -> u2loc, slot-major (contiguous per partition)
                    for q in range(NQ):
                        nc.sync.dma_start(
                            out=u2loc[q].rearrange("(a b) h -> a b h", b=BPC),
                            in_=u2panel[q * PS : (q + 1) * PS, :].rearrange(
                                "p (b h) -> p b h", h=P
                            ),
                        )

                tc.strict_bb_all_engine_barrier()
                run_cc = stop_after not in ("A", "B")
                if run_b and stop_after == "B":
                    with tc.tile_pool(name="dbgB", bufs=1) as dbg:
                        z = dbg.tile([OUT_C, SHARD], dt.float32, tag="zB")
                        nc.vector.memset(z[:], 0)
                        nc.sync.dma_start(out=y.ap(), in_=z[:])
                for q in range(NQ if run_cc else 0):
                    nc.gpsimd.collective_compute(
                        "AllGather",
                        mybir.AluOpType.bypass,
                        replica_groups=[list(range(NCORES))],
                        ins=[u2loc[q]],
                        outs=[T2q[q][:]],
                    )
                tc.strict_bb_all_engine_barrier()
                run_d = stop_after not in ("A", "B", "CC")
                if run_cc and stop_after == "CC":
                    with tc.tile_pool(name="dbgC", bufs=1) as dbg:
                        z = dbg.tile([OUT_C, SHARD], dt.float32, tag="zC")
                        nc.vector.memset(z[:], 0)
                        nc.sync.dma_start(out=y.ap(), in_=z[:])

                # ---------------- phase D: layer-2 aggregation + FC --------------
                if run_d:
                  with (
                    tc.tile_pool(name="phD", bufs=GBUFS) as pD,
                    tc.tile_pool(name="phDs", bufs=2) as pDs,
                    tc.tile_pool(name="phDe", bufs=3) as pDe,
                    tc.tile_pool(name="psD", bufs=2, space="PSUM") as psD,
                    tc.tile_pool(name="psDa", bufs=3, space="PSUM") as psDa,
                    tc.tile_pool(name="ypl", bufs=2) as ypool,
                ):
                    # y is written in quarter panels to bound SBUF footprint
                    HB = (BPC + 3) // 4
                    ndb = int(os.environ.get("KB_D_BLOCKS", BPC))
                    ystate = {}

                    def epi2(b, psum_agg):
                        if b % HB == 0:
                            ypanel = ypool.tile(
                                [OUT_C, HB * P], dt.float32, tag="ypanel"
                            )
                            ystate["t"] = ypanel
                            ystate["b0"] = b
                        h2 = pDe.tile([P, OUT_C], dt.float32, tag="h2")
                        nc.scalar.activation(
                            out=h2[:], in_=psum_agg[:], func=AF.Copy,
                            scale=gconst["dinv_blk"][:, b : b + 1],
                        )
                        ptr = psD.tile([OUT_C, P], dt.float32, space="PSUM", tag="tr2")
                        nc.tensor.transpose(out=ptr[:], in_=h2[:], identity=ident[:])
                        h2t = pDe.tile([OUT_C, P], dt.bfloat16, tag="h2t")
                        nc.scalar.copy(out=h2t[:], in_=ptr[:])
                        py = psD.tile([OUT_C, P], dt.float32, space="PSUM", tag="py")
                        nc.tensor.matmul(
                            out=py[:], lhsT=gconst["Wfc"][:], rhs=h2t[:],
                            start=True, stop=True,
                        )
                        b0 = ystate["b0"]
                        nc.scalar.activation(
                            out=ystate["t"][:, (b - b0) * P : (b - b0 + 1) * P],
                            in_=py[:],
                            func=AF.Identity, bias=gconst["bfc_col"][:, 0:1],
                        )
                        if b % HB == HB - 1 or b == ndb - 1:
                            nc.sync.dma_start(
                                out=y.ap()[:, b0 * P : (b + 1) * P],
                                in_=ystate["t"][:, : (b - b0 + 1) * P],
                            )

                    agg_pass(pD, pDs, psDa, gconst["iota2"],
                             [T2q[q][:] for q in range(NQ)],
                             K2, gidx2_t, gconst["dstloc2"][:], P, OUT_C,
                             gconst["b2row"][:], epi2, nblocks=ndb)

    nc.compile()
    return nc


def _make_runner(nc):
    """jit-compiled SPMD runner over 8 cores (mirrors bass2jax.run_bass_via_pjrt
    but reusable across calls so executions can be timed warm)."""
    import jax
    import numpy as np
    from jax.sharding import Mesh, PartitionSpec
    from jax.experimental.shard_map import shard_map
    import concourse.mybir as mybir
    from concourse import bass2jax

    bass2jax.install_neuronx_cc_hook()
    partition_name = nc.partition_id_tensor.name if nc.partition_id_tensor else None
    in_names, out_names, out_avals, zero_outs = [], [], [], []
    for alloc in nc.m.functions[0].allocations:
        if not isinstance(alloc, mybir.MemoryLocationSet):
            continue
        name = alloc.memorylocations[0].name
        if alloc.kind == "ExternalInput":
            if name != partition_name:
                in_names.append(name)
        elif alloc.kind == "ExternalOutput":
            out_names.append(name)
            shape = tuple(alloc.tensor_shape)
            dtype = mybir.dt.np(alloc.dtype)
            out_avals.append(jax.core.ShapedArray(shape, dtype))
            zero_outs.append(np.zeros(shape, dtype))
    n_params = len(in_names)
    all_in_names = list(in_names) + list(out_names)
    if partition_name is not None:
        all_in_names.append(partition_name)

    def _body(*args):
        operands = list(args)
        if partition_name is not None:
            operands.append(bass2jax.partition_id_tensor())
        outs = bass2jax._bass_exec_p.bind(
            *operands,
            out_avals=tuple(out_avals),
            in_names=tuple(all_in_names),
            out_names=tuple(out_names),
            lowering_input_output_aliases=(),
            sim_require_finite=True,
            sim_require_nnan=True,
            nc=nc,
        )
        return tuple(outs)

    devices = jax.devices()[:NCORES]
    mesh = Mesh(np.asarray(devices), ("core",))
    in_specs = (PartitionSpec("core"),) * (n_params + len(out_names))
    out_specs = (PartitionSpec("core"),) * len(out_names)
    fn = jax.jit(
        shard_map(_body, mesh=mesh, in_specs=in_specs, out_specs=out_specs,
                  check_rep=False),
        keep_unused=True,
    )
    return fn, in_names, out_names, zero_outs, mesh


def kernel(x, edge_index, W1, b1, W2, b2, Wfc, bfc, _trace=False, _bench=True):
    import time as _time
    import jax
    from jax.sharding import NamedSharding, PartitionSpec

    import os as _os
    K1, K2, in_maps, perm_pos = _preprocess(x, edge_index, W1, b1, W2, b2, Wfc, bfc)
    key = (K1, K2, _os.environ.get("KB_REPEAT", "1"),
           _os.environ.get("KB_STOP_AFTER", ""), _os.environ.get("KB_D_BLOCKS", ""),
           _os.environ.get("KB_B_BLOCKS", ""), _os.environ.get("KB_SHARED_T2", "0"),
           _os.environ.get("KB_GATHER_ONLY", "0"))
    if key not in _kernel_cache:
        nc = _build(K1, K2)
        _kernel_cache[key] = (nc, _make_runner(nc))
    nc, (fn, in_names, out_names, zero_outs, mesh) = _kernel_cache[key]

    sh = NamedSharding(mesh, PartitionSpec("core"))
    concat_in = [
        np.concatenate([np.asarray(in_maps[c][nm]) for c in range(NCORES)], axis=0)
        for nm in in_names
    ]
    concat_zeros = [
        np.zeros((NCORES * z.shape[0], *z.shape[1:]), z.dtype) for z in zero_outs
    ]
    dev_in = [jax.device_put(a, sh) for a in concat_in + concat_zeros]
    out_arrs = fn(*dev_in)
    jax.block_until_ready(out_arrs)

    if _bench:
        times = []
        for _ in range(5):
            t0 = _time.perf_counter()
            out_arrs = fn(*dev_in)
            jax.block_until_ready(out_arrs)
            times.append(_time.perf_counter() - t0)
        kernel._last_times = times
        kernel._last_exec_time_ns = int(min(times) * 1e9)
    else:
        kernel._last_exec_time_ns = None
    if not hasattr(kernel, "_runners"):
        kernel._runners = {}
    kernel._runners[_os.environ.get("KB_REPEAT", "1")] = (fn, dev_in)

    outs = {nm: np.asarray(out_arrs[i]) for i, nm in enumerate(out_names)}
    Y = (
        outs["y"].reshape(NCORES, OUT_C, SHARD)
        .transpose(0, 2, 1)
        .reshape(NCORES * SHARD, OUT_C)
    )
    return Y[perm_pos[:N]].astype(np.float32)
